# revision 25
# baseline (speedup 1.0000x reference)
"""Trainium2 Bass kernel for nn_Block_67637144977876 (sparse_attention).

Self-contained: accepts FULL inputs, shards across 8 NeuronCores
(data-parallel: core = one batch x one 32-row image band = 4096 tokens
= 16 complete 16x16 windows), runs one SPMD NEFF, gathers the output.

Per-core layout is channel-major ([ch, tok]); attention scores are
computed transposed ([key, tok]) so no on-chip transposes are needed.
512-token chunks (one window pair); score PSUM tiles hold a HEAD PAIR
(2 banks, [128, 1024]) so each ScalarE exp covers two heads.
Relative-position bias is applied multiplicatively
(exp(s+b) = exp(s)*exp(b)) with host-precomputed exp(b) multiplied in
on GpSimd. The window noise-add is folded into the q/k/v projections
as a rank-1 K=1 matmul (colsum(W) x noise-row). Pixel-norm rsqrt is
exp(-0.5*ln(m)) so the whole attention path shares one activation
table set (Bacc's table picker is steered via a patched table list).
The residual stream is bf16 held in one in-place buffer; output is
bf16, cast to fp32 on the host.
"""
import numpy as np
import ml_dtypes

import concourse.bacc as bacc
import concourse.tile as tile
from concourse import mybir
from concourse.bass_utils import run_bass_kernel_spmd
from concourse.dve_ops import AFFINE_THEN_ADD
from concourse.hw_specs import get_activation_tables as _real_gat

_EXPLN_SET = "natural_log_exp_and_others"


def _gat_patched(arch):
    """Table list for Bacc.insert_act_table_loads, with Exp/Ln visible only
    in the one set that serves both — the greedy first-match picker would
    otherwise bounce between exp_and_others and natural_log every window.
    Key order (= canonical act_func_set_id) is unchanged."""
    exp = mybir.ActivationFunctionType.Exp
    ln = mybir.ActivationFunctionType.Ln
    out = {}
    for name, funcs in _real_gat(arch).items():
        if name != _EXPLN_SET:
            funcs = funcs - {exp, ln}
        out[name] = funcs
    return out


bacc.get_activation_tables = _gat_patched

F32 = mybir.dt.float32
BF16 = mybir.dt.bfloat16
BF = ml_dtypes.bfloat16

DIM = 256
HEADS = 8
HD = 32
WS = 16
BS = 2
HW = 128
N = HW * HW
EN = 256
HID = 4 * DIM
NCORE = 8
TOK = 4096          # tokens per core
TC = 512            # token chunk (= one window pair)
NTC = TOK // TC
SCALE = HD ** -0.5
KREP = int(__import__("os").environ.get("KREP", "1"))
GELU_GATE = __import__("os").environ.get("GELU_GATE", "0") == "1"
PS2B = int(__import__("os").environ.get("PS2B", "2"))
PS1B = int(__import__("os").environ.get("PS1B", "2"))
PSFB = int(__import__("os").environ.get("PSFB", "2"))
WBUF = int(__import__("os").environ.get("WBUF", "2"))
EBDVE = __import__("os").environ.get("EBDVE", "0") == "1"
SQDVE = __import__("os").environ.get("SQDVE", "0") == "1"

_NC_CACHE = None


def _rel_pos_index():
    c = np.stack(np.meshgrid(np.arange(WS), np.arange(WS), indexing="ij"))
    c = c.reshape(2, -1)
    rel = c[:, :, None] - c[:, None, :]
    rel = rel.transpose(1, 2, 0) + (WS - 1)
    return rel[..., 0] * (2 * WS - 1) + rel[..., 1]


def _perm():
    """t' (window-major) -> n (row-major within the core's 32x128 slab)."""
    t = np.arange(TOK)
    win, intra = t // 256, t % 256
    wr, wc = win // 8, win % 8
    rr, cc = intra // 16, intra % 16
    return (wr * 16 + rr) * 128 + (wc * 16 + cc)


def build_nc():
    nc = bacc.Bacc("TRN2", debug=False)
    dt = nc.dram_tensor
    xT16 = dt("xT16", (DIM, TOK), BF16, kind="ExternalInput")
    embT = dt("embT", (DIM, EN), BF16, kind="ExternalInput")
    nrow16 = dt("nrow16", (1, TOK), BF16, kind="ExternalInput")
    csum = dt("csum", (1, 3 * DIM), BF16, kind="ExternalInput")
    cqw = dt("cqw", (DIM, DIM), BF16, kind="ExternalInput")
    ckw = dt("ckw", (DIM, DIM), BF16, kind="ExternalInput")
    cvw = dt("cvw", (DIM, DIM), BF16, kind="ExternalInput")
    cpw = dt("cpw", (DIM, DIM), BF16, kind="ExternalInput")
    qw = dt("qw", (DIM, DIM), BF16, kind="ExternalInput")
    kw = dt("kw", (DIM, DIM), BF16, kind="ExternalInput")
    vw = dt("vw", (DIM, DIM), BF16, kind="ExternalInput")
    apw = dt("apw", (DIM, DIM), BF16, kind="ExternalInput")
    f1w = dt("f1w", (DIM, HID), BF16, kind="ExternalInput")
    f2w = dt("f2w", (HID, DIM), BF16, kind="ExternalInput")
    ebias = dt("ebias", (128, HEADS * 2 * 512), BF16, kind="ExternalInput")
    cpb = dt("cpb", (DIM, 1), F32, kind="ExternalInput")
    apb = dt("apb", (DIM, 1), F32, kind="ExternalInput")
    f1b = dt("f1b", (HID, 1), F32, kind="ExternalInput")
    f2b = dt("f2b", (DIM, 1), F32, kind="ExternalInput")
    ones32 = dt("ones32", (128, 32), BF16, kind="ExternalInput")
    onesn = dt("onesn", (128, 128), BF16, kind="ExternalInput")
    outT = dt("outT", (DIM, TOK), BF16, kind="ExternalOutput")

    EXP = mybir.ActivationFunctionType.Exp
    LN = mybir.ActivationFunctionType.Ln
    GELU = mybir.ActivationFunctionType.Gelu

    with tile.TileContext(nc) as tc:
        with (
            tc.tile_pool(name="wts", bufs=1) as wts,
            tc.tile_pool(name="xbuf", bufs=1) as xbuf,
            tc.tile_pool(name="work", bufs=WBUF) as work,
            tc.tile_pool(name="attn", bufs=WBUF) as attn,
            tc.tile_pool(name="ps", bufs=PS1B, space="PSUM") as ps,
            tc.tile_pool(name="ps2", bufs=PS2B, space="PSUM") as ps2,
            tc.tile_pool(name="psf", bufs=PSFB, space="PSUM") as psf,
        ):
            def pst(name):
                return ps.tile([128, TC], F32, name=name, tag="bank")

            def pst2(name):
                return ps2.tile([128, 2 * TC], F32, name=name, tag="bank2")

            def pstf(name):
                return psf.tile([128, TC], F32, name=name, tag="fbank")

            def wt(shape, name, dtype=F32, bufs=None, tag=None):
                kw_ = {"bufs": bufs} if bufs else {}
                return work.tile(list(shape), dtype, name=name,
                                 tag=tag or name, **kw_)

            # ---- resident loads (row-chunked to 128 partitions) ----
            # round-robin across engine HWDGE queues so transfers overlap
            _dmaengs = [nc.sync, nc.scalar, nc.gpsimd]
            _dmaidx = [0]

            def _dma(out_, in_):
                eng = _dmaengs[_dmaidx[0] % len(_dmaengs)]
                _dmaidx[0] += 1
                eng.dma_start(out=out_, in_=in_)

            def load(t, shape, dtype=BF16):
                rows, cols = shape
                if rows <= 128:
                    s = wts.tile([rows, cols], dtype, name=f"sb_{t.name}")
                    _dma(s, t.ap())
                    return s
                out = []
                for i in range(rows // 128):
                    s = wts.tile([128, cols], dtype, name=f"sb_{t.name}{i}")
                    _dma(s, t.ap()[128 * i:128 * (i + 1), :])
                    out.append(s)
                return out

            # x residual stream, bf16, loaded in place (doubles as stage-2
            # input and x2 storage); load order = first-use order
            xb = [xbuf.tile([128, TOK], BF16, name=f"xb{m}") for m in range(2)]
            nc.sync.dma_start(out=xb[0], in_=xT16.ap()[0:128, :])
            nc.scalar.dma_start(out=xb[1], in_=xT16.ap()[128:256, :])
            s_embT = load(embT, (DIM, EN))
            s_cqw = load(cqw, (DIM, DIM))
            s_ckw = load(ckw, (DIM, DIM))
            s_cvw = load(cvw, (DIM, DIM))
            s_cpw = load(cpw, (DIM, DIM))
            s_o32 = load(ones32, (128, 32))
            s_on = load(onesn, (128, 128))
            s_cpb = load(cpb, (DIM, 1), F32)
            s_qw = load(qw, (DIM, DIM))
            s_kw = load(kw, (DIM, DIM))
            s_vw = load(vw, (DIM, DIM))
            s_apw = load(apw, (DIM, DIM))
            s_apb = load(apb, (DIM, 1), F32)
            s_csum = load(csum, (1, 3 * DIM))
            s_bias = load(ebias, (128, HEADS * 2 * 512))
            s_f1w = load(f1w, (DIM, HID))
            s_f2w = load(f2w, (HID, DIM))
            s_f1b = load(f1b, (HID, 1), F32)
            s_f2b = load(f2b, (DIM, 1), F32)
            s_eps = wts.tile([128, 1], F32, name="eps")
            nc.vector.memset(s_eps, 1e-8)

            s2buf = xbuf.tile([128, TOK], BF16, name="s2buf")

            def emit_pipeline():
                # ---- cross-attn K/V prep ----
                k_cm = [wts.tile([128, EN], BF16, name=f"kcm{m}") for m in range(2)]
                v_km = [wts.tile([128, DIM], BF16, name=f"vkm{m}") for m in range(2)]
                for m in range(2):
                    p = pst(f"kv_ps{m}")
                    for kc in range(2):
                        nc.tensor.matmul(
                            p[:, 0:EN],
                            s_ckw[kc][:, 128 * m:128 * (m + 1)],
                            s_embT[kc],
                            start=(kc == 0), stop=(kc == 1))
                    nc.vector.tensor_copy(k_cm[m], p[:, 0:EN])
                    p2 = pst(f"vv_ps{m}")
                    for kc in range(2):
                        nc.tensor.matmul(
                            p2[:, 0:DIM],
                            s_embT[kc][:, 128 * m:128 * (m + 1)],
                            s_cvw[kc],
                            start=(kc == 0), stop=(kc == 1))
                    nc.vector.tensor_copy(v_km[m], p2[:, 0:DIM])

                # softmax tail: expt[h][kc] are [128, TC] bf16 APs; v_lhsT
                # yields (col-slice, lhsT list) pairs per head
                def attn_tail(g, expt, v_lhsT, pname):
                    sb = pst(f"{pname}sb{g}")
                    for h in range(4):
                        for kc in range(2):
                            nc.tensor.matmul(
                                sb[32 * h:32 * h + 32, :], s_o32[:, 0:32],
                                expt[h][kc], start=(kc == 0), stop=(kc == 1),
                                tile_position=(0, 32 * h))
                    rb = wt([128, TC], f"rb{g}", F32, tag=f"rb{g}")
                    nc.vector.reciprocal_approx_fast(out=rb, in_=sb)
                    ou = pst(f"{pname}ou{g}")
                    for h in range(4):
                        for csl, lhsTs in v_lhsT(h):
                            for kc in range(2):
                                nc.tensor.matmul(
                                    ou[32 * h:32 * h + 32, csl], lhsTs[kc],
                                    expt[h][kc][:, csl],
                                    start=(kc == 0), stop=(kc == 1),
                                    tile_position=(0, 32 * h))
                    on = work.tile([128, TC], BF16, name=f"on{g}", tag=f"on{g}")
                    nc.vector.tensor_mul(on, ou, rb)
                    return on

                # ---- stage 2: cross attention ----
                for t in range(NTC):
                    tsl = slice(TC * t, TC * (t + 1))
                    q_cm = [work.tile([128, TC], BF16, name=f"qcm{m}", tag=f"qcm{m}")
                            for m in range(2)]
                    for m in range(2):
                        p = pst(f"qp{m}_{t}")
                        for kc in range(2):
                            nc.tensor.matmul(
                                p, s_cqw[kc][:, 128 * m:128 * (m + 1)],
                                xb[kc][:, tsl],
                                start=(kc == 0), stop=(kc == 1))
                        nc.vector.tensor_copy(q_cm[m], p)
                    o_n = [None, None]
                    for g in range(2):
                        expt = [[None] * 2 for _ in range(4)]
                        for kc in range(2):
                            for pr in range(2):
                                sc2 = pst2(f"sc{g}{pr}{kc}_{t}")
                                for hl in range(2):
                                    h = 2 * pr + hl
                                    nc.tensor.matmul(
                                        sc2[:, TC * hl:TC * (hl + 1)],
                                        k_cm[g][32 * h:32 * h + 32, 128 * kc:128 * (kc + 1)],
                                        q_cm[g][32 * h:32 * h + 32, :],
                                        start=True, stop=True,
                                        tile_position=(32 * h, 0))
                                e2 = attn.tile([128, 2 * TC], BF16,
                                               name=f"ex{pr}{kc}",
                                               tag=f"ex{pr}{kc}")
                                nc.scalar.activation(out=e2, in_=sc2, func=EXP)
                                for hl in range(2):
                                    expt[2 * pr + hl][kc] = e2[:, TC * hl:TC * (hl + 1)]

                        def v_lhsT(h, g=g):
                            hh = 4 * g + h
                            return [(slice(0, TC),
                                     [v_km[kc][:, 32 * hh:32 * hh + 32]
                                      for kc in range(2)])]
                        o_n[g] = attn_tail(g, expt, v_lhsT, f"c{t}")
                    for m in range(2):
                        p = pst(f"cp{m}_{t}")
                        for kc in range(2):
                            nc.tensor.matmul(
                                p, s_cpw[kc][:, 128 * m:128 * (m + 1)],
                                o_n[kc], start=(kc == 0), stop=(kc == 1))
                        nc.vector._custom_dve(
                            AFFINE_THEN_ADD, out=xb[m][:, tsl], in0=p,
                            in1=xb[m][:, tsl], s0=1.0,
                            s1=s_cpb[m])

                # ---- stage 3+4: window norm + attention, per window pair ----
                for wp in range(NTC):
                    tsl = slice(TC * wp, TC * (wp + 1))
                    # pixel norm; noise is folded into qkv as rank-1 matmuls
                    sq = [work.tile([128, TC], BF16, name=f"sq{m}", tag=f"sq{m}")
                          for m in range(2)]
                    for m in range(2):
                        eng = nc.vector if SQDVE else nc.gpsimd
                        eng.tensor_mul(sq[m], xb[m][:, tsl], xb[m][:, tsl])
                    mb = pst(f"mb_{wp}")
                    for kc in range(2):
                        nc.tensor.matmul(mb, s_on, sq[kc],
                                         start=(kc == 0), stop=(kc == 1))
                    zl = wt([128, TC], "zl", BF16, bufs=1)
                    nc.scalar.activation(out=zl, in_=mb, func=LN, bias=s_eps)
                    sbc = wt([128, TC], "sbc", BF16, bufs=1)
                    nc.scalar.activation(out=sbc, in_=zl, func=EXP, scale=-0.5)
                    nb16 = wt([1, TC], "nb16", BF16, bufs=2)
                    nc.sync.dma_start(out=nb16, in_=nrow16.ap()[:, tsl])
                    xn = [work.tile([128, TC], BF16, name=f"xn{m}", tag=f"xn{m}")
                          for m in range(2)]
                    for m in range(2):
                        nc.vector.tensor_mul(xn[m], xb[m][:, tsl], sbc)
                    # q/k projections (channel-major); noise enters as a
                    # rank-1 K=1 matmul: colsum(W) (x) noise-row
                    qk = {}
                    for m in range(2):
                        for wi, (wname, w) in enumerate((("q", s_qw), ("k", s_kw))):
                            p = pst(f"qk{wname}{m}_{wp}")
                            for kc in range(2):
                                nc.tensor.matmul(
                                    p, w[kc][:, 128 * m:128 * (m + 1)],
                                    xn[kc], start=(kc == 0), stop=False)
                            nc.tensor.matmul(
                                p,
                                s_csum[0:1, 256 * wi + 128 * m:256 * wi + 128 * (m + 1)],
                                nb16, start=False, stop=True)
                            d = work.tile([128, TC], BF16, name=f"{wname}w{m}",
                                          tag=f"{wname}w{m}")
                            nc.vector.tensor_copy(d, p)
                            qk[(wname, m)] = d
                    # v projections, key-major; 2 windows packed along free dim
                    vtile = [None, None]
                    for kcw in range(2):
                        p = pst(f"vw{kcw}_{wp}")
                        for wloc in range(2):
                            base = 256 * wloc + 128 * kcw
                            osl = slice(256 * wloc, 256 * (wloc + 1))
                            for cc in range(2):
                                nc.tensor.matmul(
                                    p[:, osl], xn[cc][:, base:base + 128],
                                    s_vw[cc],
                                    start=(cc == 0), stop=False)
                            nc.tensor.matmul(
                                p[:, osl], nb16[0:1, base:base + 128],
                                s_csum[0:1, 512:768],
                                start=False, stop=True)
                        v = attn.tile([128, TC], BF16, name=f"vkm{kcw}",
                                      tag=f"vkm{kcw}")
                        nc.vector.tensor_copy(v, p)
                        vtile[kcw] = v
                    # scores + multiplicative rel-pos bias + softmax + av
                    o_n = [None, None]
                    for g in range(2):
                        expt = [[None] * 2 for _ in range(4)]
                        for kc in range(2):
                            for pr in range(2):
                                sc2 = pst2(f"wsc{g}{pr}{kc}_{wp}")
                                for hl in range(2):
                                    h = 2 * pr + hl
                                    for wloc in range(2):
                                        kbase = 256 * wloc + 128 * kc
                                        qbase = 256 * wloc
                                        nc.tensor.matmul(
                                            sc2[:, TC * hl + 256 * wloc:TC * hl + 256 * (wloc + 1)],
                                            qk[("k", g)][32 * h:32 * h + 32, kbase:kbase + 128],
                                            qk[("q", g)][32 * h:32 * h + 32, qbase:qbase + 256],
                                            start=True, stop=True,
                                            tile_position=(32 * h, 0))
                                e2 = attn.tile([128, 2 * TC], BF16,
                                               name=f"ex{pr}{kc}",
                                               tag=f"ex{pr}{kc}")
                                nc.scalar.activation(out=e2, in_=sc2, func=EXP)
                                bsl = slice(1024 * (4 * g + 2 * pr + kc),
                                            1024 * (4 * g + 2 * pr + kc + 1))
                                if EBDVE:
                                    nc.vector.tensor_mul(e2, e2, s_bias[:, bsl])
                                else:
                                    nc.gpsimd.tensor_mul(e2, e2, s_bias[:, bsl])
                                for hl in range(2):
                                    expt[2 * pr + hl][kc] = e2[:, TC * hl:TC * (hl + 1)]

                        def v_lhsT(h, g=g):
                            hh = 4 * g + h
                            return [
                                (slice(256 * wloc, 256 * (wloc + 1)),
                                 [vtile[kc][:, 256 * wloc + 32 * hh:
                                             256 * wloc + 32 * hh + 32]
                                  for kc in range(2)])
                                for wloc in range(2)
                            ]
                        o_n[g] = attn_tail(g, expt, v_lhsT, f"w{wp}")
                    for m in range(2):
                        p = pst(f"ap{m}_{wp}")
                        for kc in range(2):
                            nc.tensor.matmul(
                                p, s_apw[kc][:, 128 * m:128 * (m + 1)],
                                o_n[kc], start=(kc == 0), stop=(kc == 1))
                        nc.vector._custom_dve(
                            AFFINE_THEN_ADD, out=xb[m][:, tsl], in0=p,
                            in1=xb[m][:, tsl], s0=1.0,
                            s1=s_apb[m])
                    msq = [work.tile([128, TC], BF16, name=f"msq{m}",
                                     tag=f"sq{m}") for m in range(2)]
                    for m in range(2):
                        eng = nc.vector if SQDVE else nc.gpsimd
                        eng.tensor_mul(msq[m], xb[m][:, tsl],
                                       xb[m][:, tsl])
                    mb2 = pst(f"mb2_{wp}")
                    for kc in range(2):
                        nc.tensor.matmul(mb2, s_on, msq[kc],
                                         start=(kc == 0), stop=(kc == 1))
                    zl2 = wt([128, TC], "zl2", BF16, bufs=1)
                    nc.scalar.activation(out=zl2, in_=mb2, func=LN, bias=s_eps)
                    nc.scalar.activation(out=s2buf[:, tsl], in_=zl2,
                                         func=EXP, scale=-0.5)

                # gate: gelu (own act-table set) can wait for the last
                # window chunk so the exp/ln set isn't thrashed mid-stream
                if GELU_GATE:
                    gz = wt([128, 1], "gz", F32, bufs=1)
                    nc.vector.tensor_scalar_mul(gz, s2buf[:, TOK - 1:TOK], 0.0)
                    f1bg = []
                    for hc in range(8):
                        t_ = wt([128, 1], f"f1bg{hc}", F32, bufs=1)
                        nc.vector.tensor_add(t_, s_f1b[hc], gz)
                        f1bg.append(t_)
                else:
                    f1bg = s_f1b

                # ---- stage 5: MLP (x2 lives in xb buffers) ----
                for t in range(NTC):
                    tsl = slice(TC * t, TC * (t + 1))
                    xn2 = [work.tile([128, TC], BF16, name=f"xn2{m}", tag=f"xn{m}")
                           for m in range(2)]
                    for m in range(2):
                        nc.vector.tensor_mul(xn2[m], xb[m][:, tsl],
                                             s2buf[:, tsl])
                    hsb = []
                    for hc in range(8):
                        p = pstf(f"f1{hc}_{t}")
                        for kc in range(2):
                            nc.tensor.matmul(
                                p, s_f1w[kc][:, 128 * hc:128 * (hc + 1)],
                                xn2[kc], start=(kc == 0), stop=(kc == 1))
                        hh = work.tile([128, TC], BF16, name=f"h{hc}", tag=f"h{hc}",
                                       bufs=1)
                        nc.scalar.activation(
                            out=hh, in_=p, func=GELU,
                            bias=f1bg[hc])
                        hsb.append(hh)
                    for m in range(2):
                        p = pstf(f"f2{m}_{t}")
                        for hc in range(8):
                            nc.tensor.matmul(
                                p, s_f2w[hc][:, 128 * m:128 * (m + 1)],
                                hsb[hc], start=(hc == 0), stop=(hc == 7))
                        xo = wt([128, TC], f"xo{m}", BF16, tag=f"xo{m}")
                        nc.vector._custom_dve(
                            AFFINE_THEN_ADD, out=xo, in0=p,
                            in1=xb[m][:, tsl], s0=1.0,
                            s1=s_f2b[m])
                        nc.sync.dma_start(
                            out=outT.ap()[128 * m:128 * (m + 1), tsl], in_=xo)

            for _rep in range(KREP):
                emit_pipeline()

    nc.compile()
    return nc


def _host_prep(x, embeddings, noise, cq_w, ck_w, cv_w, cp_w, cp_b,
               qkv_w, ap_w, ap_b, rpb_table, noise_strength,
               fc1_w, fc1_b, fc2_w, fc2_b):
    perm = _perm()
    idx = _rel_pos_index()
    bias = np.asarray(rpb_table)[idx.reshape(-1)].reshape(
        WS * WS, WS * WS, HEADS)
    biasT = bias.transpose(2, 1, 0)  # [h, key(m), tok(n)]
    # pair-major layout: 1024-col block per (g, head-pair, kc) holding the
    # two heads' exp(bias) side by side, each duplicated over both windows
    ebias = np.zeros((128, HEADS * 2 * 512), np.float32)
    for g in range(2):
        for pr in range(2):
            for kc in range(2):
                base = 1024 * (4 * g + 2 * pr + kc)
                for hl in range(2):
                    h = 4 * g + 2 * pr + hl
                    blk = np.exp(biasT[h, 128 * kc:128 * (kc + 1), :])
                    ebias[:, base + 512 * hl:base + 512 * (hl + 1)] = (
                        np.concatenate([blk, blk], axis=1))
    f = np.asarray
    qw_s = f(qkv_w)[:, 0:DIM] * SCALE
    kw_ = f(qkv_w)[:, DIM:2 * DIM]
    vw_ = f(qkv_w)[:, 2 * DIM:3 * DIM]
    csum = np.concatenate(
        [qw_s.sum(0), kw_.sum(0), vw_.sum(0)]).reshape(1, 3 * DIM)
    shared = {
        "csum": csum.astype(BF),
        "cqw": (f(cq_w) * SCALE).astype(BF),
        "ckw": f(ck_w).astype(BF),
        "cvw": f(cv_w).astype(BF),
        "cpw": f(cp_w).astype(BF),
        "qw": qw_s.astype(BF),
        "kw": kw_.astype(BF),
        "vw": vw_.astype(BF),
        "apw": f(ap_w).astype(BF),
        "f1w": f(fc1_w).astype(BF),
        "f2w": f(fc2_w).astype(BF),
        "ebias": ebias.astype(BF),
        "cpb": f(cp_b).reshape(DIM, 1).astype(np.float32),
        "apb": f(ap_b).reshape(DIM, 1).astype(np.float32),
        "f1b": f(fc1_b).reshape(HID, 1).astype(np.float32),
        "f2b": f(fc2_b).reshape(DIM, 1).astype(np.float32),
        "ones32": np.ones((128, 32), np.float32).astype(BF),
        "onesn": np.full((128, 128), 1.0 / DIM, np.float32).astype(BF),
    }
    ins = []
    for c in range(NCORE):
        b, j = c // 4, c % 4
        xw = np.asarray(x)[b, TOK * j:TOK * (j + 1), :][perm]
        w0 = 64 * b + 16 * j
        nr = (np.asarray(noise)[w0:w0 + 16, :, 0].reshape(1, TOK)
              * float(noise_strength)).astype(np.float32)
        m = dict(shared)
        m["embT"] = np.ascontiguousarray(np.asarray(embeddings)[b].T).astype(BF)
        m["xT16"] = np.ascontiguousarray(xw.T).astype(BF)
        m["nrow16"] = nr.astype(BF)
        ins.append(m)
    return ins, perm


def kernel(**inputs):
    global _NC_CACHE
    if _NC_CACHE is None:
        _NC_CACHE = build_nc()
    nc = _NC_CACHE
    ins, perm = _host_prep(**inputs)
    res = run_bass_kernel_spmd(nc, ins, core_ids=list(range(NCORE)))
    inv = np.empty(TOK, np.int64)
    inv[perm] = np.arange(TOK)
    out = np.zeros((BS, N, DIM), np.float32)
    for c in range(NCORE):
        b, j = c // 4, c % 4
        oc = np.asarray(res.results[c]["outT"], dtype=np.float32)
        out[b, TOK * j:TOK * (j + 1), :] = oc.T[inv]
    return out


# revision 26
# speedup vs baseline: 1.2094x; 1.2094x over previous
"""Trainium2 Bass kernel for nn_Block_67637144977876 (sparse_attention).

Self-contained: accepts FULL inputs, shards across 8 NeuronCores
(data-parallel: core = one batch x one 32-row image band = 4096 tokens
= 16 complete 16x16 windows), runs one SPMD NEFF, gathers the output.

Per-core layout is channel-major ([ch, tok]); attention scores are
computed transposed ([key, tok]) so no on-chip transposes are needed.
512-token chunks (one window pair); score PSUM tiles hold a HEAD PAIR
(2 banks, [128, 1024]) so each ScalarE exp covers two heads.
Relative-position bias is applied multiplicatively
(exp(s+b) = exp(s)*exp(b)) with host-precomputed exp(b) multiplied in
on GpSimd. The window noise-add is folded into the q/k/v projections
as a rank-1 K=1 matmul (colsum(W) x noise-row). Pixel-norm rsqrt is
exp(-0.5*ln(m)) so the whole attention path shares one activation
table set (Bacc's table picker is steered via a patched table list).
The residual stream is bf16 held in one in-place buffer; output is
bf16, cast to fp32 on the host.
"""
import numpy as np
import ml_dtypes

import concourse.bacc as bacc
import concourse.tile as tile
from concourse import mybir
from concourse.bass_utils import run_bass_kernel_spmd
from concourse.dve_ops import AFFINE_THEN_ADD
from concourse.hw_specs import get_activation_tables as _real_gat

_EXPLN_SET = "natural_log_exp_and_others"


def _gat_patched(arch):
    """Table list for Bacc.insert_act_table_loads, with Exp/Ln visible only
    in the one set that serves both — the greedy first-match picker would
    otherwise bounce between exp_and_others and natural_log every window.
    Key order (= canonical act_func_set_id) is unchanged."""
    exp = mybir.ActivationFunctionType.Exp
    ln = mybir.ActivationFunctionType.Ln
    out = {}
    for name, funcs in _real_gat(arch).items():
        if name != _EXPLN_SET:
            funcs = funcs - {exp, ln}
        out[name] = funcs
    return out


bacc.get_activation_tables = _gat_patched

F32 = mybir.dt.float32
BF16 = mybir.dt.bfloat16
BF = ml_dtypes.bfloat16

DIM = 256
HEADS = 8
HD = 32
WS = 16
BS = 2
HW = 128
N = HW * HW
EN = 256
HID = 4 * DIM
NCORE = 8
TOK = 4096          # tokens per core
TC = 512            # token chunk (= one window pair)
NTC = TOK // TC
SCALE = HD ** -0.5
KREP = int(__import__("os").environ.get("KREP", "1"))
GELU_GATE = __import__("os").environ.get("GELU_GATE", "0") == "1"
PS2B = int(__import__("os").environ.get("PS2B", "2"))
PS1B = int(__import__("os").environ.get("PS1B", "2"))
PSFB = int(__import__("os").environ.get("PSFB", "2"))
WBUF = int(__import__("os").environ.get("WBUF", "2"))
EBDVE = __import__("os").environ.get("EBDVE", "0") == "1"
SQDVE = __import__("os").environ.get("SQDVE", "0") == "1"

_NC_CACHE = None


def _rel_pos_index():
    c = np.stack(np.meshgrid(np.arange(WS), np.arange(WS), indexing="ij"))
    c = c.reshape(2, -1)
    rel = c[:, :, None] - c[:, None, :]
    rel = rel.transpose(1, 2, 0) + (WS - 1)
    return rel[..., 0] * (2 * WS - 1) + rel[..., 1]


def _perm():
    """t' (window-major) -> n (row-major within the core's 32x128 slab)."""
    t = np.arange(TOK)
    win, intra = t // 256, t % 256
    wr, wc = win // 8, win % 8
    rr, cc = intra // 16, intra % 16
    return (wr * 16 + rr) * 128 + (wc * 16 + cc)


def build_nc():
    nc = bacc.Bacc("TRN2", debug=False)
    dt = nc.dram_tensor
    xT16 = dt("xT16", (DIM, TOK), BF16, kind="ExternalInput")
    embT = dt("embT", (DIM, EN), BF16, kind="ExternalInput")
    nrow16 = dt("nrow16", (1, TOK), BF16, kind="ExternalInput")
    csum = dt("csum", (1, 3 * DIM), BF16, kind="ExternalInput")
    cqw = dt("cqw", (DIM, DIM), BF16, kind="ExternalInput")
    ckw = dt("ckw", (DIM, DIM), BF16, kind="ExternalInput")
    cvw = dt("cvw", (DIM, DIM), BF16, kind="ExternalInput")
    cpw = dt("cpw", (DIM, DIM), BF16, kind="ExternalInput")
    qw = dt("qw", (DIM, DIM), BF16, kind="ExternalInput")
    kw = dt("kw", (DIM, DIM), BF16, kind="ExternalInput")
    vw = dt("vw", (DIM, DIM), BF16, kind="ExternalInput")
    apw = dt("apw", (DIM, DIM), BF16, kind="ExternalInput")
    f1w = dt("f1w", (DIM, HID), BF16, kind="ExternalInput")
    f2w = dt("f2w", (HID, DIM), BF16, kind="ExternalInput")
    ebias = dt("ebias", (128, HEADS * 2 * 512), BF16, kind="ExternalInput")
    cpb = dt("cpb", (DIM, 1), F32, kind="ExternalInput")
    apb = dt("apb", (DIM, 1), F32, kind="ExternalInput")
    f1b = dt("f1b", (HID, 1), F32, kind="ExternalInput")
    f2b = dt("f2b", (DIM, 1), F32, kind="ExternalInput")
    ones32 = dt("ones32", (128, 32), BF16, kind="ExternalInput")
    onesn = dt("onesn", (128, 128), BF16, kind="ExternalInput")
    outT = dt("outT", (DIM, TOK), BF16, kind="ExternalOutput")

    EXP = mybir.ActivationFunctionType.Exp
    LN = mybir.ActivationFunctionType.Ln
    GELU = mybir.ActivationFunctionType.Gelu

    with tile.TileContext(nc) as tc:
        with (
            tc.tile_pool(name="wts", bufs=1) as wts,
            tc.tile_pool(name="xbuf", bufs=1) as xbuf,
            tc.tile_pool(name="work", bufs=WBUF) as work,
            tc.tile_pool(name="attn", bufs=WBUF) as attn,
            tc.tile_pool(name="ps", bufs=PS1B, space="PSUM") as ps,
            tc.tile_pool(name="ps2", bufs=PS2B, space="PSUM") as ps2,
            tc.tile_pool(name="psf", bufs=PSFB, space="PSUM") as psf,
        ):
            def pst(name):
                return ps.tile([128, TC], F32, name=name, tag="bank")

            def pst2(name):
                return ps2.tile([128, 2 * TC], F32, name=name, tag="bank2")

            def pstf(name):
                return psf.tile([128, TC], F32, name=name, tag="fbank")

            def wt(shape, name, dtype=F32, bufs=None, tag=None):
                kw_ = {"bufs": bufs} if bufs else {}
                return work.tile(list(shape), dtype, name=name,
                                 tag=tag or name, **kw_)

            # ---- resident loads (row-chunked to 128 partitions) ----
            # round-robin across engine HWDGE queues so transfers overlap
            _dmaengs = [nc.sync]
            _dmaidx = [0]

            def _dma(out_, in_):
                eng = _dmaengs[_dmaidx[0] % len(_dmaengs)]
                _dmaidx[0] += 1
                eng.dma_start(out=out_, in_=in_)

            def load(t, shape, dtype=BF16):
                rows, cols = shape
                if rows <= 128:
                    s = wts.tile([rows, cols], dtype, name=f"sb_{t.name}")
                    _dma(s, t.ap())
                    return s
                out = []
                for i in range(rows // 128):
                    s = wts.tile([128, cols], dtype, name=f"sb_{t.name}{i}")
                    _dma(s, t.ap()[128 * i:128 * (i + 1), :])
                    out.append(s)
                return out

            # x residual stream, bf16, loaded in place (doubles as stage-2
            # input and x2 storage); load order = first-use order
            xb = [xbuf.tile([128, TOK], BF16, name=f"xb{m}") for m in range(2)]
            for m in range(2):
                nc.sync.dma_start(out=xb[m], in_=xT16.ap()[128 * m:128 * (m + 1), :])
            s_embT = load(embT, (DIM, EN))
            s_cqw = load(cqw, (DIM, DIM))
            s_ckw = load(ckw, (DIM, DIM))
            s_cvw = load(cvw, (DIM, DIM))
            s_cpw = load(cpw, (DIM, DIM))
            s_o32 = load(ones32, (128, 32))
            s_on = load(onesn, (128, 128))
            s_cpb = load(cpb, (DIM, 1), F32)
            s_qw = load(qw, (DIM, DIM))
            s_kw = load(kw, (DIM, DIM))
            s_vw = load(vw, (DIM, DIM))
            s_apw = load(apw, (DIM, DIM))
            s_apb = load(apb, (DIM, 1), F32)
            s_csum = load(csum, (1, 3 * DIM))
            s_bias = load(ebias, (128, HEADS * 2 * 512))
            s_f1w = load(f1w, (DIM, HID))
            s_f2w = load(f2w, (HID, DIM))
            s_f1b = load(f1b, (HID, 1), F32)
            s_f2b = load(f2b, (DIM, 1), F32)
            s_eps = wts.tile([128, 1], F32, name="eps")
            nc.vector.memset(s_eps, 1e-8)

            s2buf = xbuf.tile([128, TOK], BF16, name="s2buf")

            def emit_pipeline():
                # ---- cross-attn K/V prep ----
                k_cm = [wts.tile([128, EN], BF16, name=f"kcm{m}") for m in range(2)]
                v_km = [wts.tile([128, DIM], BF16, name=f"vkm{m}") for m in range(2)]
                for m in range(2):
                    p = pst(f"kv_ps{m}")
                    for kc in range(2):
                        nc.tensor.matmul(
                            p[:, 0:EN],
                            s_ckw[kc][:, 128 * m:128 * (m + 1)],
                            s_embT[kc],
                            start=(kc == 0), stop=(kc == 1))
                    nc.vector.tensor_copy(k_cm[m], p[:, 0:EN])
                    p2 = pst(f"vv_ps{m}")
                    for kc in range(2):
                        nc.tensor.matmul(
                            p2[:, 0:DIM],
                            s_embT[kc][:, 128 * m:128 * (m + 1)],
                            s_cvw[kc],
                            start=(kc == 0), stop=(kc == 1))
                    nc.vector.tensor_copy(v_km[m], p2[:, 0:DIM])

                # softmax tail: expt[h][kc] are [128, TC] bf16 APs; v_lhsT
                # yields (col-slice, lhsT list) pairs per head
                def attn_tail(g, expt, v_lhsT, pname):
                    sb = pst(f"{pname}sb{g}")
                    for h in range(4):
                        for kc in range(2):
                            nc.tensor.matmul(
                                sb[32 * h:32 * h + 32, :], s_o32[:, 0:32],
                                expt[h][kc], start=(kc == 0), stop=(kc == 1),
                                tile_position=(0, 32 * h))
                    rb = wt([128, TC], f"rb{g}", F32, tag=f"rb{g}")
                    nc.vector.reciprocal_approx_fast(out=rb, in_=sb)
                    ou = pst(f"{pname}ou{g}")
                    for h in range(4):
                        for csl, lhsTs in v_lhsT(h):
                            for kc in range(2):
                                nc.tensor.matmul(
                                    ou[32 * h:32 * h + 32, csl], lhsTs[kc],
                                    expt[h][kc][:, csl],
                                    start=(kc == 0), stop=(kc == 1),
                                    tile_position=(0, 32 * h))
                    on = work.tile([128, TC], BF16, name=f"on{g}", tag=f"on{g}")
                    nc.vector.tensor_mul(on, ou, rb)
                    return on

                # ---- stage 2: cross attention ----
                for t in range(NTC):
                    tsl = slice(TC * t, TC * (t + 1))
                    q_cm = [work.tile([128, TC], BF16, name=f"qcm{m}", tag=f"qcm{m}")
                            for m in range(2)]
                    for m in range(2):
                        p = pst(f"qp{m}_{t}")
                        for kc in range(2):
                            nc.tensor.matmul(
                                p, s_cqw[kc][:, 128 * m:128 * (m + 1)],
                                xb[kc][:, tsl],
                                start=(kc == 0), stop=(kc == 1))
                        nc.vector.tensor_copy(q_cm[m], p)
                    o_n = [None, None]
                    for g in range(2):
                        expt = [[None] * 2 for _ in range(4)]
                        for kc in range(2):
                            for pr in range(2):
                                sc2 = pst2(f"sc{g}{pr}{kc}_{t}")
                                for hl in range(2):
                                    h = 2 * pr + hl
                                    nc.tensor.matmul(
                                        sc2[:, TC * hl:TC * (hl + 1)],
                                        k_cm[g][32 * h:32 * h + 32, 128 * kc:128 * (kc + 1)],
                                        q_cm[g][32 * h:32 * h + 32, :],
                                        start=True, stop=True,
                                        tile_position=(32 * h, 0))
                                e2 = attn.tile([128, 2 * TC], BF16,
                                               name=f"ex{pr}{kc}",
                                               tag=f"ex{pr}{kc}")
                                nc.scalar.activation(out=e2, in_=sc2, func=EXP)
                                for hl in range(2):
                                    expt[2 * pr + hl][kc] = e2[:, TC * hl:TC * (hl + 1)]

                        def v_lhsT(h, g=g):
                            hh = 4 * g + h
                            return [(slice(0, TC),
                                     [v_km[kc][:, 32 * hh:32 * hh + 32]
                                      for kc in range(2)])]
                        o_n[g] = attn_tail(g, expt, v_lhsT, f"c{t}")
                    for m in range(2):
                        p = pst(f"cp{m}_{t}")
                        for kc in range(2):
                            nc.tensor.matmul(
                                p, s_cpw[kc][:, 128 * m:128 * (m + 1)],
                                o_n[kc], start=(kc == 0), stop=(kc == 1))
                        nc.vector._custom_dve(
                            AFFINE_THEN_ADD, out=xb[m][:, tsl], in0=p,
                            in1=xb[m][:, tsl], s0=1.0,
                            s1=s_cpb[m])

                # ---- stage 3+4: window norm + attention, per window pair ----
                for wp in range(NTC):
                    tsl = slice(TC * wp, TC * (wp + 1))
                    # pixel norm; noise is folded into qkv as rank-1 matmuls
                    sq = [work.tile([128, TC], BF16, name=f"sq{m}", tag=f"sq{m}")
                          for m in range(2)]
                    for m in range(2):
                        eng = nc.vector if SQDVE else nc.gpsimd
                        eng.tensor_mul(sq[m], xb[m][:, tsl], xb[m][:, tsl])
                    mb = pst(f"mb_{wp}")
                    for kc in range(2):
                        nc.tensor.matmul(mb, s_on, sq[kc],
                                         start=(kc == 0), stop=(kc == 1))
                    zl = wt([128, TC], "zl", BF16, bufs=1)
                    nc.scalar.activation(out=zl, in_=mb, func=LN, bias=s_eps)
                    sbc = wt([128, TC], "sbc", BF16, bufs=1)
                    nc.scalar.activation(out=sbc, in_=zl, func=EXP, scale=-0.5)
                    nb16 = wt([1, TC], "nb16", BF16, bufs=2)
                    nc.sync.dma_start(out=nb16, in_=nrow16.ap()[:, tsl])
                    xn = [work.tile([128, TC], BF16, name=f"xn{m}", tag=f"xn{m}")
                          for m in range(2)]
                    for m in range(2):
                        nc.vector.tensor_mul(xn[m], xb[m][:, tsl], sbc)
                    # q/k projections (channel-major); noise enters as a
                    # rank-1 K=1 matmul: colsum(W) (x) noise-row
                    qk = {}
                    for m in range(2):
                        for wi, (wname, w) in enumerate((("q", s_qw), ("k", s_kw))):
                            p = pst(f"qk{wname}{m}_{wp}")
                            for kc in range(2):
                                nc.tensor.matmul(
                                    p, w[kc][:, 128 * m:128 * (m + 1)],
                                    xn[kc], start=(kc == 0), stop=False)
                            nc.tensor.matmul(
                                p,
                                s_csum[0:1, 256 * wi + 128 * m:256 * wi + 128 * (m + 1)],
                                nb16, start=False, stop=True)
                            d = work.tile([128, TC], BF16, name=f"{wname}w{m}",
                                          tag=f"{wname}w{m}")
                            nc.vector.tensor_copy(d, p)
                            qk[(wname, m)] = d
                    # v projections, key-major; 2 windows packed along free dim
                    vtile = [None, None]
                    for kcw in range(2):
                        p = pst(f"vw{kcw}_{wp}")
                        for wloc in range(2):
                            base = 256 * wloc + 128 * kcw
                            osl = slice(256 * wloc, 256 * (wloc + 1))
                            for cc in range(2):
                                nc.tensor.matmul(
                                    p[:, osl], xn[cc][:, base:base + 128],
                                    s_vw[cc],
                                    start=(cc == 0), stop=False)
                            nc.tensor.matmul(
                                p[:, osl], nb16[0:1, base:base + 128],
                                s_csum[0:1, 512:768],
                                start=False, stop=True)
                        v = attn.tile([128, TC], BF16, name=f"vkm{kcw}",
                                      tag=f"vkm{kcw}")
                        nc.vector.tensor_copy(v, p)
                        vtile[kcw] = v
                    # scores + multiplicative rel-pos bias + softmax + av
                    o_n = [None, None]
                    for g in range(2):
                        expt = [[None] * 2 for _ in range(4)]
                        for kc in range(2):
                            for pr in range(2):
                                sc2 = pst2(f"wsc{g}{pr}{kc}_{wp}")
                                for hl in range(2):
                                    h = 2 * pr + hl
                                    for wloc in range(2):
                                        kbase = 256 * wloc + 128 * kc
                                        qbase = 256 * wloc
                                        nc.tensor.matmul(
                                            sc2[:, TC * hl + 256 * wloc:TC * hl + 256 * (wloc + 1)],
                                            qk[("k", g)][32 * h:32 * h + 32, kbase:kbase + 128],
                                            qk[("q", g)][32 * h:32 * h + 32, qbase:qbase + 256],
                                            start=True, stop=True,
                                            tile_position=(32 * h, 0))
                                e2 = attn.tile([128, 2 * TC], BF16,
                                               name=f"ex{pr}{kc}",
                                               tag=f"ex{pr}{kc}")
                                nc.scalar.activation(out=e2, in_=sc2, func=EXP)
                                bsl = slice(1024 * (4 * g + 2 * pr + kc),
                                            1024 * (4 * g + 2 * pr + kc + 1))
                                if EBDVE:
                                    nc.vector.tensor_mul(e2, e2, s_bias[:, bsl])
                                else:
                                    nc.gpsimd.tensor_mul(e2, e2, s_bias[:, bsl])
                                for hl in range(2):
                                    expt[2 * pr + hl][kc] = e2[:, TC * hl:TC * (hl + 1)]

                        def v_lhsT(h, g=g):
                            hh = 4 * g + h
                            return [
                                (slice(256 * wloc, 256 * (wloc + 1)),
                                 [vtile[kc][:, 256 * wloc + 32 * hh:
                                             256 * wloc + 32 * hh + 32]
                                  for kc in range(2)])
                                for wloc in range(2)
                            ]
                        o_n[g] = attn_tail(g, expt, v_lhsT, f"w{wp}")
                    for m in range(2):
                        p = pst(f"ap{m}_{wp}")
                        for kc in range(2):
                            nc.tensor.matmul(
                                p, s_apw[kc][:, 128 * m:128 * (m + 1)],
                                o_n[kc], start=(kc == 0), stop=(kc == 1))
                        nc.vector._custom_dve(
                            AFFINE_THEN_ADD, out=xb[m][:, tsl], in0=p,
                            in1=xb[m][:, tsl], s0=1.0,
                            s1=s_apb[m])
                    msq = [work.tile([128, TC], BF16, name=f"msq{m}",
                                     tag=f"sq{m}") for m in range(2)]
                    for m in range(2):
                        eng = nc.vector if SQDVE else nc.gpsimd
                        eng.tensor_mul(msq[m], xb[m][:, tsl],
                                       xb[m][:, tsl])
                    mb2 = pst(f"mb2_{wp}")
                    for kc in range(2):
                        nc.tensor.matmul(mb2, s_on, msq[kc],
                                         start=(kc == 0), stop=(kc == 1))
                    zl2 = wt([128, TC], "zl2", BF16, bufs=1)
                    nc.scalar.activation(out=zl2, in_=mb2, func=LN, bias=s_eps)
                    nc.scalar.activation(out=s2buf[:, tsl], in_=zl2,
                                         func=EXP, scale=-0.5)

                # gate: gelu (own act-table set) can wait for the last
                # window chunk so the exp/ln set isn't thrashed mid-stream
                if GELU_GATE:
                    gz = wt([128, 1], "gz", F32, bufs=1)
                    nc.vector.tensor_scalar_mul(gz, s2buf[:, TOK - 1:TOK], 0.0)
                    f1bg = []
                    for hc in range(8):
                        t_ = wt([128, 1], f"f1bg{hc}", F32, bufs=1)
                        nc.vector.tensor_add(t_, s_f1b[hc], gz)
                        f1bg.append(t_)
                else:
                    f1bg = s_f1b

                # ---- stage 5: MLP (x2 lives in xb buffers) ----
                for t in range(NTC):
                    tsl = slice(TC * t, TC * (t + 1))
                    xn2 = [work.tile([128, TC], BF16, name=f"xn2{m}", tag=f"xn{m}")
                           for m in range(2)]
                    for m in range(2):
                        nc.vector.tensor_mul(xn2[m], xb[m][:, tsl],
                                             s2buf[:, tsl])
                    hsb = []
                    for hc in range(8):
                        p = pstf(f"f1{hc}_{t}")
                        for kc in range(2):
                            nc.tensor.matmul(
                                p, s_f1w[kc][:, 128 * hc:128 * (hc + 1)],
                                xn2[kc], start=(kc == 0), stop=(kc == 1))
                        hh = work.tile([128, TC], BF16, name=f"h{hc}", tag=f"h{hc}",
                                       bufs=1)
                        nc.scalar.activation(
                            out=hh, in_=p, func=GELU,
                            bias=f1bg[hc])
                        hsb.append(hh)
                    for m in range(2):
                        p = pstf(f"f2{m}_{t}")
                        for hc in range(8):
                            nc.tensor.matmul(
                                p, s_f2w[hc][:, 128 * m:128 * (m + 1)],
                                hsb[hc], start=(hc == 0), stop=(hc == 7))
                        xo = wt([128, TC], f"xo{m}", BF16, tag=f"xo{m}")
                        nc.vector._custom_dve(
                            AFFINE_THEN_ADD, out=xo, in0=p,
                            in1=xb[m][:, tsl], s0=1.0,
                            s1=s_f2b[m])
                        nc.sync.dma_start(
                            out=outT.ap()[128 * m:128 * (m + 1), tsl], in_=xo)

            for _rep in range(KREP):
                emit_pipeline()

    nc.compile()
    return nc


def _host_prep(x, embeddings, noise, cq_w, ck_w, cv_w, cp_w, cp_b,
               qkv_w, ap_w, ap_b, rpb_table, noise_strength,
               fc1_w, fc1_b, fc2_w, fc2_b):
    perm = _perm()
    idx = _rel_pos_index()
    bias = np.asarray(rpb_table)[idx.reshape(-1)].reshape(
        WS * WS, WS * WS, HEADS)
    biasT = bias.transpose(2, 1, 0)  # [h, key(m), tok(n)]
    # pair-major layout: 1024-col block per (g, head-pair, kc) holding the
    # two heads' exp(bias) side by side, each duplicated over both windows
    ebias = np.zeros((128, HEADS * 2 * 512), np.float32)
    for g in range(2):
        for pr in range(2):
            for kc in range(2):
                base = 1024 * (4 * g + 2 * pr + kc)
                for hl in range(2):
                    h = 4 * g + 2 * pr + hl
                    blk = np.exp(biasT[h, 128 * kc:128 * (kc + 1), :])
                    ebias[:, base + 512 * hl:base + 512 * (hl + 1)] = (
                        np.concatenate([blk, blk], axis=1))
    f = np.asarray
    qw_s = f(qkv_w)[:, 0:DIM] * SCALE
    kw_ = f(qkv_w)[:, DIM:2 * DIM]
    vw_ = f(qkv_w)[:, 2 * DIM:3 * DIM]
    csum = np.concatenate(
        [qw_s.sum(0), kw_.sum(0), vw_.sum(0)]).reshape(1, 3 * DIM)
    shared = {
        "csum": csum.astype(BF),
        "cqw": (f(cq_w) * SCALE).astype(BF),
        "ckw": f(ck_w).astype(BF),
        "cvw": f(cv_w).astype(BF),
        "cpw": f(cp_w).astype(BF),
        "qw": qw_s.astype(BF),
        "kw": kw_.astype(BF),
        "vw": vw_.astype(BF),
        "apw": f(ap_w).astype(BF),
        "f1w": f(fc1_w).astype(BF),
        "f2w": f(fc2_w).astype(BF),
        "ebias": ebias.astype(BF),
        "cpb": f(cp_b).reshape(DIM, 1).astype(np.float32),
        "apb": f(ap_b).reshape(DIM, 1).astype(np.float32),
        "f1b": f(fc1_b).reshape(HID, 1).astype(np.float32),
        "f2b": f(fc2_b).reshape(DIM, 1).astype(np.float32),
        "ones32": np.ones((128, 32), np.float32).astype(BF),
        "onesn": np.full((128, 128), 1.0 / DIM, np.float32).astype(BF),
    }
    ins = []
    for c in range(NCORE):
        b, j = c // 4, c % 4
        xw = np.asarray(x)[b, TOK * j:TOK * (j + 1), :][perm]
        w0 = 64 * b + 16 * j
        nr = (np.asarray(noise)[w0:w0 + 16, :, 0].reshape(1, TOK)
              * float(noise_strength)).astype(np.float32)
        m = dict(shared)
        m["embT"] = np.ascontiguousarray(np.asarray(embeddings)[b].T).astype(BF)
        m["xT16"] = np.ascontiguousarray(xw.T).astype(BF)
        m["nrow16"] = nr.astype(BF)
        ins.append(m)
    return ins, perm


def kernel(**inputs):
    global _NC_CACHE
    if _NC_CACHE is None:
        _NC_CACHE = build_nc()
    nc = _NC_CACHE
    ins, perm = _host_prep(**inputs)
    res = run_bass_kernel_spmd(nc, ins, core_ids=list(range(NCORE)))
    inv = np.empty(TOK, np.int64)
    inv[perm] = np.arange(TOK)
    out = np.zeros((BS, N, DIM), np.float32)
    for c in range(NCORE):
        b, j = c // 4, c % 4
        oc = np.asarray(res.results[c]["outT"], dtype=np.float32)
        out[b, TOK * j:TOK * (j + 1), :] = oc.T[inv]
    return out
